# revision 9
# baseline (speedup 1.0000x reference)
"""Trainium2 Bass kernel for an additive-attention (GAT-style) head.

Reference math (N=8192, EMB=256, NHID=64, alpha=0.2):
    h      = input @ W                               [N, 64]
    s_src  = h @ a[:64];  s_dst = h @ a[64:]         [N]
    e      = leaky_relu(s_src[:,None] + s_dst[None,:], 0.2)
    att    = softmax(where(adj > 0, e, -9e15), axis=1)
    out    = att @ h                                 [N, 64]

Key algebraic restructuring (avoids any transcendental on the NxN matrix):
    exp(lrelu(t)) = max(exp(t), exp(alpha*t))  and both branches are rank-1 in
    (i, j).  Dividing row i by exp(s_src_i) (cancels in softmax):
        tau_ij = adj_ij * max(v_j, r_i * vp_j)
    with v_j = exp(s_dst_j), vp_j = exp(alpha*s_dst_j), r_i = exp((alpha-1)*s_src_i).
    Then out_i = (tau_i: @ h) / (tau_i: @ 1).

Distribution: 1-D row partition of N across 8 cores (1024 rows each).  Each
core receives its adj shard TRANSPOSED ([8192, 1024], host-side layout prep)
so that j lives on SBUF partitions and the att@h contraction can run on the
TensorEngine without on-device transposes.  The full input (8 MB) is
replicated so every core computes the full h locally -- no collectives (v1).

Per-core pipeline over 64 j-tiles of [128, 1024]:
    gpsimd DMA (int32 -> bf16 cast)  ->  DVE tensor_scalar (mult+max fused)
    ->  DVE tensor_tensor (mask mult)  ->  PE matmul accumulate into [65, 1024]
Postlude: PE transpose, softmax normalize, DMA out.
"""

import os
import sys

sys.path.insert(0, "/opt/trn_rl_repo")

import numpy as np
from contextlib import ExitStack

import concourse.bass as bass
import concourse.mybir as mybir
import concourse.tile as tile

N = 8192
EMB = 256
NHID = 64
ALPHA = 0.2
NCORES = 8
NLOC = N // NCORES          # 1024 rows per core
NT = N // 128               # 64 j-tiles
NHE = NHID + 1              # h plus ones column (for the softmax denominator)
FP32 = mybir.dt.float32
BF16 = mybir.dt.bfloat16
I32 = mybir.dt.int32

AX = mybir.AxisListType
ALU = mybir.AluOpType
ACTF = mybir.ActivationFunctionType


class WaitSplitTileContext(tile.TileContext):
    """walrus' S3_LW (ldweights/matmul) struct accepts only ONE sync-wait
    command; Tile can emit matmuls with several.  Hoist the excess waits onto
    standalone InstEventSemaphore instructions on the same engine, inserted
    immediately before the matmul in the final scheduled order."""

    _NO_SPLIT_TYPES = (
        mybir.InstDrain,
        mybir.InstEventSemaphore,
        mybir.InstUnconditionalBranch,
        mybir.InstCall,
        mybir.InstRegisterMove,
    )

    def _add_instruction(self, inst):
        si = getattr(inst, "sync_info", None)
        if (
            si is not None
            and si.on_wait
            and len(si.on_wait) > 1
            and not isinstance(inst, self._NO_SPLIT_TYPES)
        ):
            waits = list(si.on_wait)
            for i, w in enumerate(waits[:-1]):
                ev = mybir.InstEventSemaphore(
                    name=f"{inst.name}-wsplit{i}",
                    engine=inst.engine,
                    ins=[],
                    outs=[],
                    sync_info=mybir.SyncInfo(on_wait=[w], on_update=[]),
                    bass_nofuse=True,
                )
                super()._add_instruction(ev)
            inst.sync_info = mybir.SyncInfo(
                on_wait=[waits[-1]], on_update=list(si.on_update)
            )
        super()._add_instruction(inst)

    def _drain_and_barrier(self, tick_clock, wait_clock):
        # The stock version attaches every engine's final tick as waits on ONE
        # drain (19 waits here) -- over walrus' per-instruction limit.  Compute
        # the waits on a probe instruction, emit them as single-wait
        # EventSemaphores on the sync queue, then a clean drain.
        from concourse.vector_clock import ScopedClock

        probe = mybir.InstEventSemaphore(
            name=f"drain-wsplit-probe-{self.nc.next_id()}",
            engine=mybir.EngineType.SP,
            ins=[],
            outs=[],
            sync_info=None,
            bass_nofuse=True,
        )
        wait_clock.add_sem_waits(probe, ScopedClock({None: tick_clock.global_clock}))
        waits = list(probe.sync_info.on_wait) if probe.sync_info else []
        for i, w in enumerate(waits):
            ev = mybir.InstEventSemaphore(
                name=f"drain-wsplit{i}-{self.nc.next_id()}",
                engine=mybir.EngineType.SP,
                ins=[],
                outs=[],
                sync_info=mybir.SyncInfo(on_wait=[w], on_update=[]),
                bass_nofuse=True,
            )
            self._add_instruction(ev)
        self.nc.sync.drain()

        self.nc.all_engine_barrier()
        assert self.sems is not None
        popped = self.nc._tile_sem_poison_stack.pop()
        assert popped is self._sem_poison
        self.nc.clear_and_free_semaphores(list(self.sems.allocated().values()))
        self.nc.all_engine_barrier()


def build_kernel() -> bass.Bass:
    nc = bass.Bass()

    adjT = nc.declare_dram_parameter("adjT", [N, NLOC], I32, isOutput=False)
    inputT = nc.declare_dram_parameter("inputT", [EMB, N], FP32, isOutput=False)
    inputT_loc = nc.declare_dram_parameter("inputT_loc", [EMB, NLOC], FP32, isOutput=False)
    W_p = nc.declare_dram_parameter("W", [EMB, NHID], FP32, isOutput=False)
    WT_p = nc.declare_dram_parameter("WT", [NHID, EMB], FP32, isOutput=False)
    a_p = nc.declare_dram_parameter("a", [2 * NHID], FP32, isOutput=False)
    ident_p = nc.declare_dram_parameter("ident", [128, 128], FP32, isOutput=False)
    out_p = nc.declare_dram_parameter("out", [NLOC, NHID], FP32, isOutput=True)

    with WaitSplitTileContext(nc) as tc, ExitStack() as ctx:
        const = ctx.enter_context(tc.tile_pool(name="const", bufs=1))
        ps_scr = ctx.enter_context(
            tc.tile_pool(name="ps_scr", bufs=2, space=bass.MemorySpace.PSUM)
        )
        ps_acc = ctx.enter_context(
            tc.tile_pool(name="ps_acc", bufs=1, space=bass.MemorySpace.PSUM)
        )
        adj_pool = ctx.enter_context(tc.tile_pool(name="adj", bufs=6))
        kap_pool = ctx.enter_context(tc.tile_pool(name="kap", bufs=3))
        p_pool = ctx.enter_context(tc.tile_pool(name="p", bufs=3))

        # ---- constant / preamble tiles ----
        inT = [const.tile([128, N], FP32, tag=f"inT{k}", name=f"inT{k}") for k in range(2)]
        inTl = [const.tile([128, NLOC], FP32, tag=f"inTl{k}", name=f"inTl{k}") for k in range(2)]
        w_sb = [const.tile([128, NHID], FP32, tag=f"w{k}", name=f"w{k}") for k in range(2)]
        wt_sb = const.tile([NHID, EMB], FP32)
        a_src = const.tile([NHID, 1], FP32, tag="asrc")
        a_dst = const.tile([NHID, 1], FP32, tag="adst")
        ident = const.tile([128, 128], FP32)
        ones1 = const.tile([1, 128], FP32)
        wa_src = const.tile([128, 2], FP32, tag="wasrc")
        wa_dst = const.tile([128, 2], FP32, tag="wadst")
        r_row = const.tile([1, NLOC], FP32)
        r_bc = const.tile([128, NLOC], BF16)
        v_cols = const.tile([128, NT], FP32, tag="vcols")
        vp_cols = const.tile([128, NT], FP32, tag="vpcols")
        h65 = const.tile([128, NT * NHE], BF16)
        houT = const.tile([NHE, NLOC], FP32)
        out_sb = const.tile([128, 8 * NHID], FP32)

        for k in range(2):
            nc.sync.dma_start(inT[k][:], inputT[128 * k : 128 * (k + 1), :])
            nc.sync.dma_start(inTl[k][:], inputT_loc[128 * k : 128 * (k + 1), :])
            nc.sync.dma_start(w_sb[k][:], W_p[128 * k : 128 * (k + 1), :])
        nc.sync.dma_start(wt_sb[:], WT_p[:])
        nc.sync.dma_start(a_src[:], a_p[0:NHID])
        nc.sync.dma_start(a_dst[:], a_p[NHID : 2 * NHID])
        nc.sync.dma_start(ident[:], ident_p[:])
        nc.vector.memset(ones1[:], 1.0)

        # ---- wa = W @ a_half, [256] as two [128,1] columns ----
        for half, (dst, asb) in enumerate([(wa_src, a_src), (wa_dst, a_dst)]):
            for ec in range(2):
                ps = ps_scr.tile([128, 1], FP32, tag="scr")
                nc.tensor.matmul(
                    ps[:], wt_sb[:, 128 * ec : 128 * (ec + 1)], asb[:],
                    start=True, stop=True,
                )
                nc.vector.tensor_copy(dst[:, ec : ec + 1], ps[:])

        # ---- s_src (local rows) as a [1, NLOC] row; r = exp((a-1)*s_src) ----
        for ih in range(2):
            ps = ps_scr.tile([1, 512], FP32, tag="scr")
            for kc in range(2):
                nc.tensor.matmul(
                    ps[:], wa_src[:, kc : kc + 1],
                    inTl[kc][:, 512 * ih : 512 * (ih + 1)],
                    start=(kc == 0), stop=(kc == 1),
                )
            nc.scalar.activation(
                r_row[:, 512 * ih : 512 * (ih + 1)], ps[:], ACTF.Exp,
                scale=ALPHA - 1.0,
            )

        # ---- r broadcast to all 128 partitions (ones outer product) ----
        for ih in range(2):
            ps = ps_scr.tile([128, 512], FP32, tag="scr")
            nc.tensor.matmul(
                ps[:], ones1[:], r_row[:, 512 * ih : 512 * (ih + 1)],
                start=True, stop=True,
            )
            nc.vector.tensor_copy(r_bc[:, 512 * ih : 512 * (ih + 1)], ps[:])

        # ---- s_dst for ALL rows, laid out as [128, 64] (col t = j-tile t) ----
        ps_sd = ps_acc.tile([128, NT], FP32, tag="ps_sd")
        for t in range(NT):
            for kc in range(2):
                nc.tensor.matmul(
                    ps_sd[:, t : t + 1],
                    inT[kc][:, 128 * t : 128 * (t + 1)],
                    wa_dst[:, kc : kc + 1],
                    start=(kc == 0), stop=(kc == 1),
                )
        nc.scalar.activation(v_cols[:], ps_sd[:], ACTF.Exp)
        nc.scalar.activation(vp_cols[:], ps_sd[:], ACTF.Exp, scale=ALPHA)

        # ---- h65: h for all rows in [j-part, e] layout + ones column ----
        h65_3d = h65[:].rearrange("p (t e) -> p t e", e=NHE)
        nc.vector.memset(h65_3d[:, :, NHID], 1.0)
        for g in range(8):
            ps = ps_scr.tile([128, 8 * NHID], FP32, tag="scr")
            for tt in range(8):
                t = 8 * g + tt
                for kc in range(2):
                    nc.tensor.matmul(
                        ps[:, NHID * tt : NHID * (tt + 1)],
                        inT[kc][:, 128 * t : 128 * (t + 1)],
                        w_sb[kc][:],
                        start=(kc == 0), stop=(kc == 1),
                    )
            nc.vector.tensor_copy(
                h65_3d[:, 8 * g : 8 * (g + 1), 0:NHID],
                ps[:].rearrange("p (t e) -> p t e", e=NHID),
            )

        # ---- main loop over 64 j-tiles ----
        ps_out = ps_acc.tile([NHE, NLOC], FP32, tag="ps_out")
        for t in range(NT):
            adj_bf = adj_pool.tile([128, NLOC], BF16)
            nc.gpsimd.dma_start(adj_bf[:], adjT[128 * t : 128 * (t + 1), :])
            kap = kap_pool.tile([128, NLOC], BF16)
            nc.vector.tensor_scalar(
                kap[:], r_bc[:],
                vp_cols[:, t : t + 1], v_cols[:, t : t + 1],
                ALU.mult, ALU.max,
            )
            p = p_pool.tile([128, NLOC], BF16)
            nc.vector.tensor_mul(p[:], kap[:], adj_bf[:])
            for ih in range(2):
                nc.tensor.matmul(
                    ps_out[:, 512 * ih : 512 * (ih + 1)],
                    h65_3d[:, t, :],
                    p[:, 512 * ih : 512 * (ih + 1)],
                    start=(t == 0), stop=(t == NT - 1),
                )

        # ---- normalize + transpose + store ----
        nc.vector.tensor_copy(houT[:], ps_out[:])
        for ic in range(8):
            ps_t = ps_scr.tile([128, NHE], FP32, tag="scr")
            nc.tensor.transpose(
                ps_t[:], houT[:, 128 * ic : 128 * (ic + 1)], ident[:NHE, :NHE]
            )
            zrec = kap_pool.tile([128, 1], FP32, tag="zrec")
            nc.vector.reciprocal(zrec[:], ps_t[:, NHID : NHID + 1])
            nc.vector.tensor_scalar(
                out_sb[:, NHID * ic : NHID * (ic + 1)], ps_t[:, 0:NHID],
                zrec[:], None, ALU.mult,
            )
        nc.sync.dma_start(
            out_p[:].rearrange("(c p) e -> p c e", p=128),
            out_sb[:].rearrange("p (c e) -> p c e", e=NHID),
        )

    return nc


def shard_inputs(input, adj, W, a):
    """Host-side sharding/layout prep. Returns in_maps for the 8 cores."""
    input = np.ascontiguousarray(np.asarray(input, dtype=np.float32))
    adj = np.asarray(adj, dtype=np.int32)
    W = np.ascontiguousarray(np.asarray(W, dtype=np.float32))
    a = np.ascontiguousarray(np.asarray(a, dtype=np.float32))
    inputT = np.ascontiguousarray(input.T)
    WT = np.ascontiguousarray(W.T)
    ident = np.eye(128, dtype=np.float32)
    in_maps = []
    for c in range(NCORES):
        rows = slice(c * NLOC, (c + 1) * NLOC)
        in_maps.append(
            {
                "adjT": np.ascontiguousarray(adj[rows, :].T),
                "inputT": inputT,
                "inputT_loc": np.ascontiguousarray(inputT[:, rows]),
                "W": W,
                "WT": WT,
                "a": a,
                "ident": ident,
            }
        )
    return in_maps


_CACHE = {}


def kernel(input, adj, W, a, _trace=False, _return_result=False):
    from concourse.bass_utils import run_bass_kernel_spmd

    if "nc" not in _CACHE:
        _CACHE["nc"] = build_kernel()
    nc = _CACHE["nc"]
    in_maps = shard_inputs(input, adj, W, a)
    res = run_bass_kernel_spmd(
        nc, in_maps, core_ids=list(range(NCORES)), trace=_trace
    )
    out = np.concatenate([res.results[c]["out"] for c in range(NCORES)], axis=0)
    if _return_result:
        return out, res
    return out


if __name__ == "__main__":
    rng = np.random.default_rng(0)
    inp = rng.standard_normal((N, EMB), dtype=np.float32)
    adj = rng.integers(0, 2, size=(N, N), dtype=np.int32)
    W = (rng.standard_normal((EMB, NHID)) * 0.05).astype(np.float32)
    a = (rng.standard_normal(2 * NHID) * 0.05).astype(np.float32)
    out = kernel(inp, adj, W, a)
    print(out.shape, out.dtype)


# revision 13
# speedup vs baseline: 1.1655x; 1.1655x over previous
"""Trainium2 Bass kernel for an additive-attention (GAT-style) head.

Reference math (N=8192, EMB=256, NHID=64, alpha=0.2):
    h      = input @ W                               [N, 64]
    s_src  = h @ a[:64];  s_dst = h @ a[64:]         [N]
    e      = leaky_relu(s_src[:,None] + s_dst[None,:], 0.2)
    att    = softmax(where(adj > 0, e, -9e15), axis=1)
    out    = att @ h                                 [N, 64]

Key algebraic restructuring (no transcendental ever touches the NxN matrix):
    exp(lrelu(t)) = max(exp(t), exp(alpha*t)) and both branches are rank-1 in
    (i, j).  Dividing row i by exp(s_src_i) (cancels in softmax) and factoring
    v_j = exp(s_dst_j) into the matmul lhs:
        tau_ij = adj_ij * v_j * max(1, r_i * w_j)
    with r_i = exp((alpha-1)*s_src_i), w_j = exp((alpha-1)*s_dst_j).
    Then out_i = (tau_i: @ h) / (tau_i: @ 1), where v_j*h_j and v_j*1 are the
    columns of the pre-scaled gathered payload.

Distribution: 1-D row partition of N across 8 cores (1024 rows each).  Each
core gets its adj shard TRANSPOSED ([8192, 1024] int32, host-side layout prep)
so j lives on SBUF partitions and the att@h contraction runs on TensorEngine
without on-device transposes.  Each core computes h (and s_dst, via an extra
W@a_dst column appended to W) for its OWN rows only, scales rows by v, and an
AllGather distributes the [N, 66] payload (v*h | v | s_dst) to every core.

Per-core main loop over 64 j-tiles of [128, 1024]:
    gpsimd DMA (int32 -> bf16 cast)  ->  DVE tensor_scalar (mult+max fused)
    ->  DVE tensor_tensor (mask mult)  ->  PE matmul accumulate into [65, 1024]
Postlude: PE transpose, softmax normalize, DMA out.
"""

import os
import sys

sys.path.insert(0, "/opt/trn_rl_repo")

import numpy as np
from contextlib import ExitStack

import concourse.bass as bass
import concourse.mybir as mybir
import concourse.tile as tile

N = 8192
EMB = 256
NHID = 64
ALPHA = 0.2
NCORES = 8
NLOC = N // NCORES          # 1024 rows per core
NT = N // 128               # 64 j-tiles
NHE = NHID + 1              # h plus ones column (for the softmax denominator)
FP32 = mybir.dt.float32
BF16 = mybir.dt.bfloat16
I32 = mybir.dt.int32

AX = mybir.AxisListType
ALU = mybir.AluOpType
ACTF = mybir.ActivationFunctionType


class WaitSplitTileContext(tile.TileContext):
    """walrus' S3_LW (ldweights/matmul) struct accepts only ONE sync-wait
    command; Tile can emit matmuls with several.  Hoist the excess waits onto
    standalone InstEventSemaphore instructions on the same engine, inserted
    immediately before the matmul in the final scheduled order."""

    _NO_SPLIT_TYPES = (
        mybir.InstDrain,
        mybir.InstEventSemaphore,
    )

    def _add_instruction(self, inst):
        si = getattr(inst, "sync_info", None)
        if (
            si is not None
            and si.on_wait
            and len(si.on_wait) > 1
            and not isinstance(inst, self._NO_SPLIT_TYPES)
        ):
            waits = list(si.on_wait)
            for i, w in enumerate(waits[:-1]):
                ev = mybir.InstEventSemaphore(
                    name=f"{inst.name}-wsplit{i}",
                    engine=inst.engine,
                    ins=[],
                    outs=[],
                    sync_info=mybir.SyncInfo(on_wait=[w], on_update=[]),
                    bass_nofuse=True,
                )
                super()._add_instruction(ev)
            inst.sync_info = mybir.SyncInfo(
                on_wait=[waits[-1]], on_update=list(si.on_update)
            )
        super()._add_instruction(inst)

    def _drain_and_barrier(self, tick_clock, wait_clock):
        # The stock version attaches every engine's final tick as waits on ONE
        # drain (19 waits here) -- over walrus' per-instruction limit.  Compute
        # the waits on a probe instruction, emit them as single-wait
        # EventSemaphores on the sync queue, then a clean drain.
        from concourse.vector_clock import ScopedClock

        probe = mybir.InstEventSemaphore(
            name=f"drain-wsplit-probe-{self.nc.next_id()}",
            engine=mybir.EngineType.SP,
            ins=[],
            outs=[],
            sync_info=None,
            bass_nofuse=True,
        )
        wait_clock.add_sem_waits(probe, ScopedClock({None: tick_clock.global_clock}))
        waits = list(probe.sync_info.on_wait) if probe.sync_info else []
        for i, w in enumerate(waits):
            ev = mybir.InstEventSemaphore(
                name=f"drain-wsplit{i}-{self.nc.next_id()}",
                engine=mybir.EngineType.SP,
                ins=[],
                outs=[],
                sync_info=mybir.SyncInfo(on_wait=[w], on_update=[]),
                bass_nofuse=True,
            )
            self._add_instruction(ev)
        self.nc.sync.drain()

        self.nc.all_engine_barrier()
        assert self.sems is not None
        popped = self.nc._tile_sem_poison_stack.pop()
        assert popped is self._sem_poison
        self.nc.clear_and_free_semaphores(list(self.sems.allocated().values()))
        self.nc.all_engine_barrier()


def build_kernel() -> bass.Bass:
    nc = bass.Bass(num_devices=NCORES)

    adjT = nc.declare_dram_parameter("adjT", [N, NLOC], I32, isOutput=False)
    inputT_loc = nc.declare_dram_parameter("inputT_loc", [EMB, NLOC], FP32, isOutput=False)
    W_p = nc.declare_dram_parameter("W", [EMB, NHID], FP32, isOutput=False)
    WT_p = nc.declare_dram_parameter("WT", [NHID, EMB], FP32, isOutput=False)
    a_p = nc.declare_dram_parameter("a", [2 * NHID], FP32, isOutput=False)
    ident_p = nc.declare_dram_parameter("ident", [128, 128], FP32, isOutput=False)
    out_p = nc.declare_dram_parameter("out", [NLOC, NHID], FP32, isOutput=True)

    NPC = NHID + 2              # payload cols: v*h (64) | v (1) | s_dst (1)
    ag_in = nc.dram_tensor("ag_in", [NLOC, NPC], FP32)
    ag_out = nc.dram_tensor("ag_out", [N, NPC], FP32, addr_space="Shared")

    with WaitSplitTileContext(nc) as tc, ExitStack() as ctx:
        const = ctx.enter_context(tc.tile_pool(name="const", bufs=1))
        ps_scr = ctx.enter_context(
            tc.tile_pool(name="ps_scr", bufs=2, space=bass.MemorySpace.PSUM)
        )
        ps_acc = ctx.enter_context(
            tc.tile_pool(name="ps_acc", bufs=1, space=bass.MemorySpace.PSUM)
        )
        adj_pool = ctx.enter_context(tc.tile_pool(name="adj", bufs=6))
        kap_pool = ctx.enter_context(tc.tile_pool(name="kap", bufs=3))
        p_pool = ctx.enter_context(tc.tile_pool(name="p", bufs=3))

        # ---- constant / preamble tiles ----
        inTl = [const.tile([128, NLOC], BF16, tag=f"inTl{k}", name=f"inTl{k}") for k in range(2)]
        w_sb = [const.tile([128, NHID], FP32, tag=f"w{k}", name=f"w{k}") for k in range(2)]
        wext = [const.tile([128, NHE], BF16, tag=f"wext{k}", name=f"wext{k}") for k in range(2)]
        wt_sb = const.tile([NHID, EMB], FP32)
        a_src = const.tile([NHID, 1], FP32, tag="asrc")
        a_dst = const.tile([NHID, 1], FP32, tag="adst")
        ident = const.tile([128, 128], FP32)
        ones1 = const.tile([1, 128], BF16)
        wa_src = const.tile([128, 2], BF16, tag="wasrc")
        r_row = const.tile([1, NLOC], BF16)
        r_bc = const.tile([128, NLOC], BF16)
        w_cols = const.tile([128, NT], FP32, tag="wcols")
        pay = const.tile([128, 8 * NPC], FP32)
        h66f = const.tile([128, NT * NPC], FP32)
        h66 = const.tile([128, NT * NPC], BF16)
        houT = const.tile([NHE, NLOC], FP32)
        out_sb = const.tile([128, 8 * NHID], FP32)

        # bf16 cast during DMA needs SWDGE (gpsimd)
        for k in range(2):
            nc.gpsimd.dma_start(inTl[k][:], inputT_loc[128 * k : 128 * (k + 1), :])
            nc.sync.dma_start(w_sb[k][:], W_p[128 * k : 128 * (k + 1), :])
        nc.sync.dma_start(wt_sb[:], WT_p[:])
        nc.sync.dma_start(a_src[:], a_p[0:NHID])
        nc.sync.dma_start(a_dst[:], a_p[NHID : 2 * NHID])
        nc.sync.dma_start(ident[:], ident_p[:])
        nc.vector.memset(ones1[:], 1.0)

        # ---- wa = W @ a_half for src and dst halves ----
        for half, (asb, tag) in enumerate([(a_src, "s"), (a_dst, "d")]):
            for ec in range(2):
                ps = ps_scr.tile([128, 1], FP32, tag="scr", name=f"ps_wa{half}{ec}")
                nc.tensor.matmul(
                    ps[:], wt_sb[:, 128 * ec : 128 * (ec + 1)], asb[:],
                    start=True, stop=True,
                )
                if half == 0:
                    nc.vector.tensor_copy(wa_src[:, ec : ec + 1], ps[:])
                else:
                    # W_ext column 64 = W @ a_dst (bf16)
                    nc.vector.tensor_copy(wext[ec][:, NHID : NHID + 1], ps[:])
        for ec in range(2):
            nc.vector.tensor_copy(wext[ec][:, 0:NHID], w_sb[ec][:])

        # ---- s_src (local rows) row; r = exp((a-1)*s_src), broadcast ----
        for ih in range(2):
            ps = ps_scr.tile([1, 512], FP32, tag="scr", name=f"ps_ss{ih}")
            for kc in range(2):
                nc.tensor.matmul(
                    ps[:], wa_src[:, kc : kc + 1],
                    inTl[kc][:, 512 * ih : 512 * (ih + 1)],
                    start=(kc == 0), stop=(kc == 1),
                )
            nc.scalar.activation(
                r_row[:, 512 * ih : 512 * (ih + 1)], ps[:], ACTF.Exp,
                scale=ALPHA - 1.0,
            )
        for ih in range(2):
            ps = ps_scr.tile([128, 512], FP32, tag="scr", name=f"ps_rb{ih}")
            nc.tensor.matmul(
                ps[:], ones1[:], r_row[:, 512 * ih : 512 * (ih + 1)],
                start=True, stop=True,
            )
            nc.vector.tensor_copy(r_bc[:, 512 * ih : 512 * (ih + 1)], ps[:])

        # ---- local h66 payload: (v*h | v | s_dst) for this core's rows ----
        pay3 = pay[:].rearrange("p (c e) -> p c e", e=NPC)
        for ic in range(8):
            ps = ps_scr.tile([128, NHE], FP32, tag="scr", name=f"ps_h{ic}")
            for kc in range(2):
                nc.tensor.matmul(
                    ps[:],
                    inTl[kc][:, 128 * ic : 128 * (ic + 1)],
                    wext[kc][:],
                    start=(kc == 0), stop=(kc == 1),
                )
            vloc = kap_pool.tile([128, 1], FP32, tag="vloc", name=f"vloc{ic}")
            nc.scalar.activation(vloc[:], ps[:, NHID : NHID + 1], ACTF.Exp)
            nc.vector.tensor_scalar(
                pay3[:, ic, 0:NHID], ps[:, 0:NHID], vloc[:], None, ALU.mult
            )
            nc.vector.tensor_copy(pay3[:, ic, NHID : NHID + 1], vloc[:])
            nc.vector.tensor_copy(pay3[:, ic, NHID + 1 : NHID + 2], ps[:, NHID : NHID + 1])

        # ---- AllGather the payload; readback in [j-tile, col] SBUF layout ----
        ag_sem = nc.alloc_semaphore("ag_sem")
        with tc.tile_critical():
            nc.sync.dma_start(
                ag_in[:].rearrange("(c p) e -> p c e", p=128), pay3
            ).then_inc(ag_sem, 16)
            nc.gpsimd.wait_ge(ag_sem, 16)
            nc.gpsimd.collective_compute(
                "AllGather",
                ALU.bypass,
                replica_groups=[list(range(NCORES))],
                ins=[ag_in[:]],
                outs=[ag_out[:]],
            ).then_inc(ag_sem)
            nc.sync.wait_ge(ag_sem, 17)
            nc.sync.dma_start(
                h66f[:].rearrange("p (t e) -> p t e", e=NPC),
                ag_out[:].rearrange("(t p) e -> p t e", p=128),
            ).then_inc(ag_sem, 16)
            nc.sync.wait_ge(ag_sem, 33)
        nc.vector.tensor_copy(h66[:], h66f[:])

        h66_3d = h66[:].rearrange("p (t e) -> p t e", e=NPC)
        # w_cols = exp((alpha-1) * s_dst_all) from payload col 65 (strided)
        nc.scalar.activation(w_cols[:], h66_3d[:, :, NHID + 1], ACTF.Exp, scale=ALPHA - 1.0)

        # ---- main loop over 64 j-tiles ----
        ps_out = ps_acc.tile([NHE, NLOC], FP32, tag="ps_out")
        for t in range(NT):
            adj_bf = adj_pool.tile([128, NLOC], BF16)
            nc.gpsimd.dma_start(adj_bf[:], adjT[128 * t : 128 * (t + 1), :])
            kap = kap_pool.tile([128, NLOC], BF16)
            nc.vector.tensor_scalar(
                kap[:], r_bc[:],
                w_cols[:, t : t + 1], 1.0,
                ALU.mult, ALU.max,
            )
            p = p_pool.tile([128, NLOC], BF16)
            nc.vector.tensor_mul(p[:], kap[:], adj_bf[:])
            for ih in range(2):
                nc.tensor.matmul(
                    ps_out[:, 512 * ih : 512 * (ih + 1)],
                    h66_3d[:, t, 0:NHE],
                    p[:, 512 * ih : 512 * (ih + 1)],
                    start=(t == 0), stop=(t == NT - 1),
                )

        # ---- normalize + transpose + store ----
        nc.vector.tensor_copy(houT[:], ps_out[:])
        for ic in range(8):
            ps_t = ps_scr.tile([128, NHE], FP32, tag="scr", name=f"ps_t{ic}")
            nc.tensor.transpose(
                ps_t[:], houT[:, 128 * ic : 128 * (ic + 1)], ident[:NHE, :NHE]
            )
            zrec = kap_pool.tile([128, 1], FP32, tag="zrec", name=f"zrec{ic}")
            nc.vector.reciprocal(zrec[:], ps_t[:, NHID : NHID + 1])
            nc.vector.tensor_scalar(
                out_sb[:, NHID * ic : NHID * (ic + 1)], ps_t[:, 0:NHID],
                zrec[:], None, ALU.mult,
            )
        nc.sync.dma_start(
            out_p[:].rearrange("(c p) e -> p c e", p=128),
            out_sb[:].rearrange("p (c e) -> p c e", e=NHID),
        )

    return nc


def shard_inputs(input, adj, W, a):
    """Host-side sharding/layout prep. Returns in_maps for the 8 cores."""
    input = np.ascontiguousarray(np.asarray(input, dtype=np.float32))
    adj = np.asarray(adj, dtype=np.int32)
    W = np.ascontiguousarray(np.asarray(W, dtype=np.float32))
    a = np.ascontiguousarray(np.asarray(a, dtype=np.float32))
    inputT = np.ascontiguousarray(input.T)
    WT = np.ascontiguousarray(W.T)
    ident = np.eye(128, dtype=np.float32)
    in_maps = []
    for c in range(NCORES):
        rows = slice(c * NLOC, (c + 1) * NLOC)
        in_maps.append(
            {
                "adjT": np.ascontiguousarray(adj[rows, :].T),
                "inputT_loc": np.ascontiguousarray(inputT[:, rows]),
                "W": W,
                "WT": WT,
                "a": a,
                "ident": ident,
            }
        )
    return in_maps


_CACHE = {}


def kernel(input, adj, W, a, _trace=False, _return_result=False):
    from concourse.bass_utils import run_bass_kernel_spmd

    if "nc" not in _CACHE:
        _CACHE["nc"] = build_kernel()
    nc = _CACHE["nc"]
    in_maps = shard_inputs(input, adj, W, a)
    res = run_bass_kernel_spmd(
        nc, in_maps, core_ids=list(range(NCORES)), trace=_trace
    )
    out = np.concatenate([res.results[c]["out"] for c in range(NCORES)], axis=0)
    if _return_result:
        return out, res
    return out


if __name__ == "__main__":
    rng = np.random.default_rng(0)
    inp = rng.standard_normal((N, EMB), dtype=np.float32)
    adj = rng.integers(0, 2, size=(N, N), dtype=np.int32)
    W = (rng.standard_normal((EMB, NHID)) * 0.05).astype(np.float32)
    a = (rng.standard_normal(2 * NHID) * 0.05).astype(np.float32)
    out = kernel(inp, adj, W, a)
    print(out.shape, out.dtype)
